# revision 2
# baseline (speedup 1.0000x reference)
"""DeepIRT (DKVMN) Trainium2 kernel — scan architecture, per-batch pipelined.

Contract: kernel(**inputs) takes the FULL unsharded inputs of reference.py's
setup_inputs() and returns the full [64, 500] float32 output.

Strategy (8 NeuronCores, pure data parallel over batch; BL=8 rows per core):
  phase 1: gather k/v embeddings into [128d, tok] bf16 SBUF archives
    (tok = b*512 + t; tables pre-transposed/cast to bf16, gathered directly).
  phase 2 (up-front waves, grouped by ACT function to avoid table reloads):
    e = sigmoid(vWe+be) (f32), a = tanh(vWa+ba) (f32), 1/e (bf16);
    logits^T = Mk k^T via one [50m, 512t] matmul per b; expw = exp (bf16).
  per-b pipeline (b = 0..7), emitted so phase3(b) overlaps plumbing(b+1):
    plumb(b):
      Z = sum_m expw (Pool partition-reduce), rz = 1/Z^T ([125t, 4blk] via PE)
      e/a transposed per 125-block via PE; ACT-evac'd bf16 with scale -+rz
        (folds softmax normalization into the coefficients):
        lhs_g[b][blk] = [ones_row; -e^T/Z rows]   [126, 128]
        lhs_a[b][blk] = [zeros_row; a^T/Z rows]
        (rows 1..125 DMA'd in with +1 partition shift)
      expw written to w_dram [b, m, 500] bf16
    phase 3(b): for each m (2-m groups):
      rhs = [ones_row; diag(expw[b,m,blk])] via diagonal-scatter DMAs into
        persistent zeroed staging tiles (only diag positions rewritten)
      g = lhs_g^T @ rhs = 1 - w*e -> PSUM f32 [128, 500]    (4 matmuls)
      h = lhs_a^T @ rhs = w*a     -> PSUM, ACT-evac'd bf16  (4 matmuls)
      traj[m] = tensor_tensor_scan(data0=g[PSUM], data1=h, init=Mv0T[:,m])
        on DVE: one instruction runs all 500 steps, fp32 internal state
      Pool TT-tree reduces traj over m into S[b][d, 1+t] = sum_m Mv_t
    reads(b) on Pool: read_t = (S[t-1] - S[t] + a_t) * (1/e_t)
    phase 4(b): f = tanh([reads, k] Wf + bf); heads; sigmoid(3*stu - qd).
"""

import numpy as np

import concourse.bass as bass
import concourse.bacc as bacc
import concourse.tile as tile
import concourse.mybir as mybir
from concourse.masks import make_identity

F32 = mybir.dt.float32
BF16 = mybir.dt.bfloat16
I32 = mybir.dt.int32
I16 = mybir.dt.int16
OP = mybir.AluOpType
AF = mybir.ActivationFunctionType

NUM_CONCEPT = 1000
D = 128
M = 50
B_FULL, T = 64, 500
NCORES = 8
BL = B_FULL // NCORES          # 8 batch rows per core
T_PAD = 512
NTOK = T_PAD * BL              # 4096 padded tokens, b-major: tok = b*T_PAD + t
TB = 125                       # scan time block
NB = 4                         # blocks per scan
TS = TB * NB                   # 500 = T exactly


def _ap(t, offset, dims):
    return bass.AP(t.tensor, offset, [list(d) for d in dims])


def build_program(debug_taps=False):
    nc = bacc.Bacc("TRN2", target_bir_lowering=False, debug=False)

    h = {}
    h["concept_seq"] = nc.declare_dram_parameter("concept_seq", [BL, T], I32, isOutput=False)
    h["correct_seq"] = nc.declare_dram_parameter("correct_seq", [BL, T], I32, isOutput=False)
    h["embed_key"] = nc.declare_dram_parameter("embed_key", [NUM_CONCEPT, D], F32, isOutput=False)
    h["embed_value"] = nc.declare_dram_parameter("embed_value", [2 * NUM_CONCEPT, D], F32, isOutput=False)
    h["Mk"] = nc.declare_dram_parameter("Mk", [M, D], F32, isOutput=False)
    h["Mv0"] = nc.declare_dram_parameter("Mv0", [M, D], F32, isOutput=False)
    h["We"] = nc.declare_dram_parameter("We", [D, D], F32, isOutput=False)
    h["be"] = nc.declare_dram_parameter("be", [D], F32, isOutput=False)
    h["Wa"] = nc.declare_dram_parameter("Wa", [D, D], F32, isOutput=False)
    h["ba"] = nc.declare_dram_parameter("ba", [D], F32, isOutput=False)
    h["Wf"] = nc.declare_dram_parameter("Wf", [2 * D, D], F32, isOutput=False)
    h["bf"] = nc.declare_dram_parameter("bf", [D], F32, isOutput=False)
    h["Wab"] = nc.declare_dram_parameter("Wab", [D, 1], F32, isOutput=False)
    h["bab"] = nc.declare_dram_parameter("bab", [1], F32, isOutput=False)
    h["Wd"] = nc.declare_dram_parameter("Wd", [D, 1], F32, isOutput=False)
    h["bd"] = nc.declare_dram_parameter("bd", [1], F32, isOutput=False)
    out_h = nc.declare_dram_parameter("out", [BL, T], F32, isOutput=True)
    dbg = {}
    if debug_taps:
        dbg["dbg_S"] = nc.declare_dram_parameter("dbg_S", [128, BL * (TS + 1)], F32, isOutput=True)
        dbg["dbg_reads"] = nc.declare_dram_parameter("dbg_reads", [128, NTOK], BF16, isOutput=True)

    with tile.TileContext(nc) as tc:
        _emit(nc, tc, h, out_h, dbg)
    nc.finalize()
    return nc


def _emit(nc, tc, h, out_h, dbg=None):
    from contextlib import ExitStack

    ctx = ExitStack()
    with ctx:
        persist = ctx.enter_context(tc.tile_pool(name="persist", bufs=1))
        dram = ctx.enter_context(tc.tile_pool(name="dram", bufs=1, space="DRAM"))

        # unnormalized softmax weights, [b, m, TS] bf16
        w_dram = dram.tile([BL, M, TS], BF16, name="w_dram")
        w_flat = w_dram.rearrange("b m t -> (b m t)")

        # persistent SBUF archives ([d, token] layouts, t-major tokens)
        k_T = persist.tile([128, NTOK], BF16)
        v_T = persist.tile([128, NTOK], BF16)
        e_T = persist.tile([128, NTOK], F32)
        erecip_T = persist.tile([128, NTOK], BF16)
        a_T = persist.tile([128, NTOK], F32)
        reads_T = persist.tile([128, NTOK], BF16)
        f_T = persist.tile([128, NTOK], BF16)

        ident = persist.tile([128, 128], F32)
        make_identity(nc, ident)

        We_f32 = persist.tile([128, 128], F32)
        nc.scalar.dma_start(out=We_f32, in_=h["We"][:, :])
        We_sb = persist.tile([128, 128], BF16)
        nc.scalar.copy(out=We_sb, in_=We_f32)
        Wa_f32 = persist.tile([128, 128], F32)
        nc.scalar.dma_start(out=Wa_f32, in_=h["Wa"][:, :])
        Wa_sb = persist.tile([128, 128], BF16)
        nc.scalar.copy(out=Wa_sb, in_=Wa_f32)
        Wf_r32 = persist.tile([128, 128], F32)
        nc.scalar.dma_start(out=Wf_r32, in_=h["Wf"][0:128, :])
        Wf_r = persist.tile([128, 128], BF16)
        nc.scalar.copy(out=Wf_r, in_=Wf_r32)
        Wf_k32 = persist.tile([128, 128], F32)
        nc.scalar.dma_start(out=Wf_k32, in_=h["Wf"][128:256, :])
        Wf_k = persist.tile([128, 128], BF16)
        nc.scalar.copy(out=Wf_k, in_=Wf_k32)
        Wab_sb = persist.tile([128, 1], F32)
        nc.sync.dma_start(out=Wab_sb, in_=h["Wab"][:, :])
        Wd_sb = persist.tile([128, 1], F32)
        nc.sync.dma_start(out=Wd_sb, in_=h["Wd"][:, :])
        Mk_sb = persist.tile([50, 128], F32)
        nc.sync.dma_start(out=Mk_sb, in_=h["Mk"][:, :])
        Mv0_sb = persist.tile([50, 128], F32)
        nc.sync.dma_start(out=Mv0_sb, in_=h["Mv0"][:, :])

        def col(name, n=128):
            t = persist.tile([n, 1], F32, name=name)
            nc.sync.dma_start(out=t, in_=_ap(h[name[:-4]][:], 0, [[1, n], [1, 1]]))
            return t

        be_col = col("be_col")
        ba_col = col("ba_col")
        bf_col = col("bf_col")

        # ---- transpose Mv0 and Mk once ----
        Mv0T_sb = persist.tile([128, 50], F32)
        MkT_sb = persist.tile([128, 50], BF16)
        S_init = persist.tile([128, 1], F32)
        with tc.tile_pool(name="init_ps", bufs=1, space="PSUM") as initp:
            mv0t = initp.tile([128, 50], F32)
            nc.tensor.transpose(mv0t, Mv0_sb, ident[0:50, 0:50])
            nc.scalar.copy(out=Mv0T_sb, in_=mv0t)
            mkt_ps = initp.tile([128, 50], F32)
            nc.tensor.transpose(mkt_ps, Mk_sb, ident[0:50, 0:50])
            nc.scalar.copy(out=MkT_sb, in_=mkt_ps)
        nc.vector.tensor_reduce(out=S_init, in_=Mv0T_sb,
                                axis=mybir.AxisListType.X, op=OP.add)

        # rhs staging: two persistent tiles, zero/ones written ONCE; the
        # diagonal scatter rewrites the same positions each group. The
        # scatter's AP makes the dep tracker attribute a byte range extending
        # ~one stage past the written region -> dead pad tile after each.
        rhs_stage = []
        for i in range(2):
            st = persist.tile([128, 8, NB, TB], BF16, name=f"rhsst{i}")
            pad = persist.tile([128, 4000], BF16, name=f"rhspad{i}")
            rhs_stage.append((st, pad))

        # lhsT tiles (persistent, rows 1..125 DMA-built per-b)
        lhs_g = [[persist.tile([128, 128], BF16, name=f"lhsg{b}_{i}") for i in range(NB)]
                 for b in range(BL)]

        # expw (unnormalized softmax numerators), [50m, b, 512t] bf16
        expwT = persist.tile([50, BL, T_PAD], BF16)

        # =========== phase 1: indices, on-chip table gathers ===========
        idxk_dram = dram.tile([NTOK], I16)
        idxv_dram = dram.tile([NTOK], I16)
        with tc.tile_pool(name="ph1", bufs=1) as ph1, \
             tc.tile_pool(name="ph1t", bufs=5) as ph1t, \
             tc.tile_pool(name="ph1ps", bufs=3, space="PSUM") as ph1ps:

            cseq = ph1.tile([8, T_PAD], I32)
            crse = ph1.tile([8, T_PAD], I32)
            nc.vector.memset(cseq, 0)
            nc.vector.memset(crse, 0)
            nc.sync.dma_start(out=cseq[:, 0:T], in_=h["concept_seq"][:, :])
            nc.scalar.dma_start(out=crse[:, 0:T], in_=h["correct_seq"][:, :])

            cseq_f = ph1.tile([8, T_PAD], F32)
            nc.vector.tensor_copy(out=cseq_f, in_=cseq)
            crse_f = ph1.tile([8, T_PAD], F32)
            nc.vector.tensor_copy(out=crse_f, in_=crse)
            x_f = ph1.tile([8, T_PAD], F32)
            nc.vector.scalar_tensor_tensor(out=x_f, in0=crse_f, scalar=float(NUM_CONCEPT),
                                           in1=cseq_f, op0=OP.mult, op1=OP.add)
            ck16s = ph1.tile([8, T_PAD], I16)
            nc.vector.tensor_copy(out=ck16s, in_=cseq)
            xv16s = ph1.tile([8, T_PAD], I16)
            nc.vector.tensor_copy(out=xv16s, in_=x_f)

            G16 = NTOK // 16
            ck16 = ph1.tile([128, G16], I16)
            xv16 = ph1.tile([128, G16], I16)
            for srct, drt, dstt, eng in ((ck16s, idxk_dram, ck16, nc.sync),
                                         (xv16s, idxv_dram, xv16, nc.scalar)):
                eng.dma_start(out=_ap(drt[:], 0, [[T_PAD, 8], [1, T_PAD]]), in_=srct)
                for j in range(8):
                    eng.dma_start(out=dstt[16 * j:16 * (j + 1), :],
                                  in_=_ap(drt[:], 0, [[1, 16], [16, G16]]))

            # tables transposed into [d, row] layout (f32: ap_gather needs
            # 4-byte elements)
            ekt = ph1.tile([128, NUM_CONCEPT], F32)
            evt = ph1.tile([128, 2 * NUM_CONCEPT], F32)
            gi = 0
            for tbl, dst_t, nrows in ((h["embed_key"], ekt, NUM_CONCEPT),
                                      (h["embed_value"], evt, 2 * NUM_CONCEPT)):
                for g0 in range(0, nrows, 512):
                    gn = min(512, nrows - g0)
                    nq = (gn + 127) // 128
                    rows4 = ph1t.tile([128, 512], F32, tag="rows4")
                    eng = (nc.sync, nc.gpsimd, nc.scalar)[gi % 3]
                    gi += 1
                    full = gn // 128
                    if full:
                        eng.dma_start(
                            out=rows4[:, 0:full * 128].rearrange("p (q c) -> p q c", c=128),
                            in_=_ap(tbl[:], g0 * 128,
                                    [[128, 128], [128 * 128, full], [1, 128]]))
                    if gn % 128:
                        rem = gn % 128
                        eng.dma_start(
                            out=rows4[0:rem, full * 128:(full + 1) * 128],
                            in_=tbl[g0 + full * 128:g0 + gn, :])
                    for q in range(nq):
                        n = min(128, gn - q * 128)
                        tps = ph1ps.tile([128, 128], F32, tag="tps")
                        nc.tensor.transpose(tps[:, 0:n],
                                            rows4[0:n, q * 128:(q + 1) * 128],
                                            ident[0:n, 0:n])
                        nc.scalar.copy(out=dst_t[:, g0 + q * 128:g0 + q * 128 + n],
                                       in_=tps[:, 0:n])

            gat = ph1.tile([128, NTOK], F32, name="gat")
            gatk = ph1.tile([128, NTOK], F32, name="gatk")
            HT = NTOK // 2
            for hf in range(2):
                hsl = slice(hf * HT, (hf + 1) * HT)
                isl = slice(hf * (HT // 16), (hf + 1) * (HT // 16))
                nc.gpsimd.ap_gather(
                    out_ap=gat[:, hsl].rearrange("p (n d) -> p n d", d=1),
                    in_ap=evt.rearrange("p (n d) -> p n d", d=1),
                    idxs_ap=xv16[:, isl], channels=128,
                    num_elems=2 * NUM_CONCEPT, d=1, num_idxs=HT)
                nc.vector.tensor_copy(out=v_T[:, hsl], in_=gat[:, hsl])
                nc.gpsimd.ap_gather(
                    out_ap=gatk[:, hsl].rearrange("p (n d) -> p n d", d=1),
                    in_ap=ekt.rearrange("p (n d) -> p n d", d=1),
                    idxs_ap=ck16[:, isl], channels=128,
                    num_elems=NUM_CONCEPT, d=1, num_idxs=HT)
                nc.vector.tensor_copy(out=k_T[:, hsl], in_=gatk[:, hsl])

        # deferred staging init (keeps the Pool queue clear during phase 1)
        for st, pad in rhs_stage:
            nc.gpsimd.memset(st, 0.0)
            nc.gpsimd.memset(st[0:1, :, :, :], 1.0)
            nc.gpsimd.memset(pad[0:1, 0:1], 0.0)  # keep pad allocated
        rhs_stage = [st for st, _ in rhs_stage]
        for b in range(BL):
            for i in range(NB):
                nc.vector.memset(lhs_g[b][i][0:1, :], 1.0)

        # =========== phase 2 waves: e, a, 1/e, logits^T, expw ===========
        with tc.tile_pool(name="ph2ps", bufs=1, space="PSUM") as plps:
            def wave_e(cs):
                with nc.allow_low_precision(reason="bf16 1/e archive, 2e-2 tol"):
                    for c in cs:
                        sl = slice(c * 512, (c + 1) * 512)
                        elog = plps.tile([128, 512], F32, tag="elog", bufs=2)
                        nc.tensor.matmul(elog, We_sb, v_T[:, sl], start=True, stop=True)
                        nc.scalar.activation(out=e_T[:, sl], in_=elog, func=AF.Sigmoid, bias=be_col)
                        nc.vector.reciprocal(out=erecip_T[:, sl], in_=e_T[:, sl])
            def wave_a(cs):
                for c in cs:
                    sl = slice(c * 512, (c + 1) * 512)
                    alog = plps.tile([128, 512], F32, tag="elog", bufs=2)
                    nc.tensor.matmul(alog, Wa_sb, v_T[:, sl], start=True, stop=True)
                    nc.scalar.activation(out=a_T[:, sl], in_=alog, func=AF.Tanh, bias=ba_col)
            def wave_w(bs):
                for b in bs:
                    sl = slice(b * 512, (b + 1) * 512)
                    wlogT = plps.tile([50, 512], F32, tag="wlogT", bufs=2)
                    nc.tensor.matmul(wlogT, MkT_sb, k_T[:, sl], start=True, stop=True)
                    nc.scalar.activation(out=expwT[:, b, :], in_=wlogT, func=AF.Exp)
            wave_e([0]); wave_a([0]); wave_w([0])
            wave_e(range(1, 8)); wave_a(range(1, 8)); wave_w(range(1, 8))

        # =========== per-b: plumbing + scans + reads + heads ===========
        a_bt = a_T.rearrange("p (b t) -> p b t", t=T_PAD)
        er_bt = erecip_T.rearrange("p (b t) -> p b t", t=T_PAD)
        rd_bt = reads_T.rearrange("p (b t) -> p b t", t=T_PAD)
        nc.vector.memset(
            reads_T.rearrange("p (b t) -> p b t", t=T_PAD)[:, :, T:T_PAD], 0.0)

        rec = ctx.enter_context(tc.tile_pool(name="rec", bufs=2))
        rec1 = ctx.enter_context(tc.tile_pool(name="rec1", bufs=3))
        recps = ctx.enter_context(tc.tile_pool(name="recps", bufs=1, space="PSUM"))
        plps = ctx.enter_context(tc.tile_pool(name="plps", bufs=1, space="PSUM"))
        finps = ctx.enter_context(tc.tile_pool(name="finps", bufs=1, space="PSUM"))
        fin = ctx.enter_context(tc.tile_pool(name="fin", bufs=2))

        gps2 = [recps.tile([128, 512], F32, name=f"gps{i}") for i in range(2)]
        gpsP = [recps.tile([128, 2, 512], F32, name=f"gpsP{i}") for i in range(2)]

        # phase-4 head constants
        Wab0 = persist.tile([128, 2], BF16)
        nc.gpsimd.memset(Wab0, 0.0)
        nc.vector.tensor_copy(out=Wab0[:, 0:1], in_=Wab_sb)
        W0d = persist.tile([128, 2], BF16)
        nc.gpsimd.memset(W0d, 0.0)
        nc.vector.tensor_copy(out=W0d[:, 1:2], in_=Wd_sb)
        comb = persist.tile([2, 1], BF16)
        nc.gpsimd.memset(comb, -1.0)
        nc.gpsimd.memset(comb[0:1, :], 3.0)
        bias2 = persist.tile([2, 1], F32)
        nc.sync.dma_start(out=bias2[0:1, :], in_=_ap(h["bab"][:], 0, [[1, 1], [1, 1]]))
        nc.sync.dma_start(out=bias2[1:2, :], in_=_ap(h["bd"][:], 0, [[1, 1], [1, 1]]))

        gp = 0
        grp = 0
        for b in range(BL):
            sl = slice(b * 512, (b + 1) * 512)
            # ---- plumb(b) ----
            zrow = rec1.tile([1, 512], F32, tag="zrow", bufs=1)
            nc.gpsimd.tensor_reduce(out=zrow, in_=expwT[:, b, :],
                                    axis=mybir.AxisListType.C, op=OP.add)
            ztile = plps.tile([TB, 264], F32, tag="tp2", bufs=1)
            zt_ps = ztile[:, 256:264]
            for i in range(NB):
                nc.tensor.transpose(zt_ps[:, i:i + 1], zrow[:, i * TB:(i + 1) * TB],
                                    ident[0:1, 0:1])
            rz = rec1.tile([TB, 8], F32, tag="rz")
            with nc.allow_low_precision(reason="1/Z softmax scale"):
                nc.vector.reciprocal(out=rz[:, 0:NB], in_=zt_ps[:, 0:NB])
            nc.vector.tensor_scalar(out=rz[:, 4:4 + NB], in0=rz[:, 0:NB],
                                    scalar1=-1.0, scalar2=None, op0=OP.mult)

            for i in range(NB):
                t0 = b * T_PAD + i * TB
                tp2 = plps.tile([TB, 264], F32, tag="tp2", bufs=1)
                nc.tensor.transpose(tp2[:, 0:128], e_T[:, t0:t0 + TB], ident)
                ea2 = rec1.tile([TB, 128], BF16, tag="ea2")
                nc.scalar.activation(out=ea2, in_=tp2[:, 0:128],
                                     func=AF.Copy, scale=rz[:, 4 + i:5 + i])
                nc.scalar.dma_start(out=lhs_g[b][i][1:1 + TB, :], in_=ea2)
            nc.scalar.dma_start(out=_ap(w_flat[:], b * M * TS, [[TS, 50], [1, TS]]),
                              in_=expwT[:, b, 0:TS])

            # car = a/e (c shifted), D_t = car_t - car_{t+1}; y-scan init tile
            car = rec1.tile([128, T_PAD], F32, tag="car", bufs=2)
            nc.gpsimd.tensor_tensor(out=car[:, 0:T], in0=a_bt[:, b, 0:T],
                                    in1=er_bt[:, b, 0:T], op=OP.mult)
            nc.gpsimd.memset(car[:, T:T + 1], 0.0)
            Dt = rec1.tile([128, T], F32, tag="Dt", bufs=2)
            nc.gpsimd.tensor_tensor(out=Dt, in0=car[:, 0:T], in1=car[:, 1:T + 1],
                                    op=OP.subtract)
            initb = rec1.tile([128, 50], F32, tag="initb", bufs=2)
            nc.vector.tensor_scalar(out=initb, in0=Mv0T_sb, scalar1=car[:, 0:1],
                                    scalar2=None, op0=OP.subtract)

            # ---- phase 3(b) ----
            S_b = rec.tile([128, TS + 1], F32, tag="sb")
            nc.vector.scalar_tensor_tensor(out=S_b[:, 0:1], in0=car[:, 0:1],
                                           scalar=-float(M), in1=S_init,
                                           op0=OP.mult, op1=OP.add)
            for mc0 in range(0, M, 8):
                msz = min(8, M - mc0)
                rhs = rhs_stage[grp % 2]
                grp += 1
                rap = rhs[:, :, :, :]
                pstr = rap.ap[0][0]
                dst = bass.AP(rap.tensor, rap.offset + pstr,
                              [[pstr + 1, TB], [NB * TB, msz], [TB, NB]])
                src = _ap(w_flat[:], (b * M + mc0) * TS,
                          [[1, TB], [TS, msz], [TB, NB]])
                nc.scalar.dma_start(out=dst, in_=src)

                traj = rec.tile([128, 8, TS], F32, tag="traj")
                ndve = 3 if msz == 8 else 1
                for mi in range(ndve):
                    m = mc0 + mi
                    gt = gps2[gp % 2]
                    for i in range(NB):
                        nc.tensor.matmul(gt[:, i * TB:(i + 1) * TB],
                                         lhs_g[b][i][0:1 + TB, :],
                                         rhs[0:1 + TB, mi, i, :],
                                         start=True, stop=True)
                    nc.vector.tensor_tensor_scan(
                        out=traj[:, mi, :],
                        data0=gt[:, 0:TS],
                        data1=Dt,
                        initial=initb[:, m:m + 1],
                        op0=OP.mult, op1=OP.add)
                    gp += 1
                # Pool share: pairs evac'd to SBUF f32 by ACT, scans on Pool
                pg = 0
                for p0 in range(ndve, msz, 2):
                    psz = min(2, msz - p0)
                    gq = gpsP[pg % 2]
                    pg += 1
                    for mi in range(p0, p0 + psz):
                        for i in range(NB):
                            nc.tensor.matmul(gq[:, mi - p0, i * TB:(i + 1) * TB],
                                             lhs_g[b][i][0:1 + TB, :],
                                             rhs[0:1 + TB, mi, i, :],
                                             start=True, stop=True)
                    gsb = rec1.tile([128, 2, TS], F32, tag="gsb")
                    nc.scalar.copy(out=gsb[:, 0:psz, :], in_=gq[:, 0:psz, 0:TS])
                    for mi in range(p0, p0 + psz):
                        m = mc0 + mi
                        nc.gpsimd.tensor_tensor_scan(
                            out=traj[:, mi, :],
                            data0=gsb[:, mi - p0, :],
                            data1=Dt,
                            initial=initb[:, m:m + 1],
                            op0=OP.mult, op1=OP.add)

                # reduce over the group's m via TT-add tree -> S
                tap = traj[:, :, :]
                part = list(tap.ap[0])
                t4 = rec.tile([128, 4, TS], F32, tag="t4", bufs=2)
                if msz == 8:
                    nc.vector.tensor_tensor(
                        out=t4,
                        in0=_ap(tap, tap.offset, [part, [2 * TS, 4], [1, TS]]),
                        in1=_ap(tap, tap.offset + TS, [part, [2 * TS, 4], [1, TS]]),
                        op=OP.add)
                    nc.gpsimd.tensor_tensor(
                        out=t4[:, 0:2, :], in0=t4[:, 0:2, :], in1=t4[:, 2:4, :],
                        op=OP.add)
                    spart = t4[:, 0, :]
                    nc.gpsimd.tensor_tensor(out=spart, in0=spart,
                                            in1=t4[:, 1, :], op=OP.add)
                else:  # msz == 2
                    spart = t4[:, 0, :]
                    nc.gpsimd.tensor_tensor(out=spart, in0=traj[:, 0, :],
                                            in1=traj[:, 1, :], op=OP.add)
                if mc0 == 0:
                    nc.gpsimd.tensor_copy(out=S_b[:, 1:TS + 1], in_=spart)
                else:
                    nc.gpsimd.tensor_tensor(out=S_b[:, 1:TS + 1],
                                            in0=S_b[:, 1:TS + 1],
                                            in1=spart, op=OP.add)

            # ---- reads(b) on Pool ----
            d1 = rec1.tile([128, T], F32, tag="d1", bufs=1)
            nc.gpsimd.tensor_tensor(out=d1, in0=S_b[:, 0:T],
                                    in1=S_b[:, 1:T + 1], op=OP.subtract)
            nc.gpsimd.scalar_tensor_tensor(out=d1, in0=Dt, scalar=float(M),
                                           in1=d1, op0=OP.mult, op1=OP.add)
            nc.gpsimd.tensor_tensor(out=d1, in0=d1, in1=a_bt[:, b, 0:T], op=OP.add)
            nc.gpsimd.tensor_tensor(out=rd_bt[:, b, 0:T], in0=d1,
                                    in1=er_bt[:, b, 0:T], op=OP.mult)

            # ---- phase 4(b) ----
            f_ps = finps.tile([128, 512], F32, tag="fps")
            nc.tensor.matmul(f_ps, Wf_r, reads_T[:, sl], start=True, stop=False)
            nc.tensor.matmul(f_ps, Wf_k, k_T[:, sl], start=False, stop=True)
            nc.scalar.activation(out=f_T[:, sl], in_=f_ps, func=AF.Tanh, bias=bf_col)
            hp2 = finps.tile([2, 512], F32, tag="fps")
            nc.tensor.matmul(hp2, Wab0, f_T[:, sl], start=True, stop=False)
            nc.tensor.matmul(hp2, W0d, k_T[:, sl], start=False, stop=True)
            ht = fin.tile([2, 512], BF16, tag="ht")
            nc.scalar.activation(out=ht, in_=hp2, func=AF.Tanh, bias=bias2)
            lg_ps = finps.tile([1, 512], F32, tag="fps")
            nc.tensor.matmul(lg_ps, comb, ht, start=True, stop=True)
            prob_row = fin.tile([1, 512], F32, tag="prob")
            nc.scalar.activation(out=prob_row, in_=lg_ps, func=AF.Sigmoid)
            nc.scalar.dma_start(out=out_h[b:b + 1, :], in_=prob_row[0:1, 0:T])

        if dbg:
            nc.sync.dma_start(out=dbg["dbg_reads"][:, :], in_=reads_T)


_NC = None
LAST_RESULT = None


def _get_nc():
    global _NC
    if _NC is None:
        _NC = build_program()
    return _NC


def kernel(**inputs):
    global LAST_RESULT
    from concourse.bass_utils import run_bass_kernel_spmd

    nc = _get_nc()
    names = ["concept_seq", "correct_seq", "embed_key", "embed_value", "Mk", "Mv0",
             "We", "be", "Wa", "ba", "Wf", "bf", "Wab", "bab", "Wd", "bd"]
    full = {k: np.ascontiguousarray(np.asarray(inputs[k])) for k in names}
    in_maps = []
    for i in range(NCORES):
        m = dict(full)
        m["concept_seq"] = np.ascontiguousarray(full["concept_seq"][i * BL:(i + 1) * BL])
        m["correct_seq"] = np.ascontiguousarray(full["correct_seq"][i * BL:(i + 1) * BL])
        in_maps.append(m)
    res = run_bass_kernel_spmd(nc, in_maps, core_ids=list(range(NCORES)))
    LAST_RESULT = res
    return np.concatenate([res.results[i]["out"] for i in range(NCORES)], axis=0)


if __name__ == "__main__":
    nc = build_program()
    print("build ok")


# revision 6
# speedup vs baseline: 1.0120x; 1.0120x over previous
"""DeepIRT (DKVMN) Trainium2 kernel — scan architecture, per-batch pipelined.

Contract: kernel(**inputs) takes the FULL unsharded inputs of reference.py's
setup_inputs() and returns the full [64, 500] float32 output.

Strategy (8 NeuronCores, pure data parallel over batch; BL=8 rows per core):
  phase 1: gather k/v embeddings into [128d, tok] bf16 SBUF archives
    (tok = b*512 + t; tables pre-transposed/cast to bf16, gathered directly).
  phase 2 (up-front waves, grouped by ACT function to avoid table reloads):
    e = sigmoid(vWe+be) (f32), a = tanh(vWa+ba) (f32), 1/e (bf16);
    logits^T = Mk k^T via one [50m, 512t] matmul per b; expw = exp (bf16).
  per-b pipeline (b = 0..7), emitted so phase3(b) overlaps plumbing(b+1):
    plumb(b):
      Z = sum_m expw (Pool partition-reduce), rz = 1/Z^T ([125t, 4blk] via PE)
      e/a transposed per 125-block via PE; ACT-evac'd bf16 with scale -+rz
        (folds softmax normalization into the coefficients):
        lhs_g[b][blk] = [ones_row; -e^T/Z rows]   [126, 128]
        lhs_a[b][blk] = [zeros_row; a^T/Z rows]
        (rows 1..125 DMA'd in with +1 partition shift)
      expw written to w_dram [b, m, 500] bf16
    phase 3(b): for each m (2-m groups):
      rhs = [ones_row; diag(expw[b,m,blk])] via diagonal-scatter DMAs into
        persistent zeroed staging tiles (only diag positions rewritten)
      g = lhs_g^T @ rhs = 1 - w*e -> PSUM f32 [128, 500]    (4 matmuls)
      h = lhs_a^T @ rhs = w*a     -> PSUM, ACT-evac'd bf16  (4 matmuls)
      traj[m] = tensor_tensor_scan(data0=g[PSUM], data1=h, init=Mv0T[:,m])
        on DVE: one instruction runs all 500 steps, fp32 internal state
      Pool TT-tree reduces traj over m into S[b][d, 1+t] = sum_m Mv_t
    reads(b) on Pool: read_t = (S[t-1] - S[t] + a_t) * (1/e_t)
    phase 4(b): f = tanh([reads, k] Wf + bf); heads; sigmoid(3*stu - qd).
"""

import numpy as np

import concourse.bass as bass
import concourse.bacc as bacc
import concourse.tile as tile
import concourse.mybir as mybir
from concourse.masks import make_identity

F32 = mybir.dt.float32
BF16 = mybir.dt.bfloat16
I32 = mybir.dt.int32
I16 = mybir.dt.int16
OP = mybir.AluOpType
AF = mybir.ActivationFunctionType

NUM_CONCEPT = 1000
D = 128
M = 50
B_FULL, T = 64, 500
NCORES = 8
BL = B_FULL // NCORES          # 8 batch rows per core
T_PAD = 512
NTOK = T_PAD * BL              # 4096 padded tokens, b-major: tok = b*T_PAD + t
TB = 125                       # scan time block
NB = 4                         # blocks per scan
TS = TB * NB                   # 500 = T exactly


def _ap(t, offset, dims):
    return bass.AP(t.tensor, offset, [list(d) for d in dims])


def build_program(debug_taps=False):
    nc = bacc.Bacc("TRN2", target_bir_lowering=False, debug=False)

    h = {}
    h["concept_seq"] = nc.declare_dram_parameter("concept_seq", [BL, T], I32, isOutput=False)
    h["correct_seq"] = nc.declare_dram_parameter("correct_seq", [BL, T], I32, isOutput=False)
    h["embed_key"] = nc.declare_dram_parameter("embed_key", [NUM_CONCEPT, D], F32, isOutput=False)
    h["embed_value"] = nc.declare_dram_parameter("embed_value", [2 * NUM_CONCEPT, D], F32, isOutput=False)
    h["Mk"] = nc.declare_dram_parameter("Mk", [M, D], F32, isOutput=False)
    h["Mv0"] = nc.declare_dram_parameter("Mv0", [M, D], F32, isOutput=False)
    h["We"] = nc.declare_dram_parameter("We", [D, D], F32, isOutput=False)
    h["be"] = nc.declare_dram_parameter("be", [D], F32, isOutput=False)
    h["Wa"] = nc.declare_dram_parameter("Wa", [D, D], F32, isOutput=False)
    h["ba"] = nc.declare_dram_parameter("ba", [D], F32, isOutput=False)
    h["Wf"] = nc.declare_dram_parameter("Wf", [2 * D, D], F32, isOutput=False)
    h["bf"] = nc.declare_dram_parameter("bf", [D], F32, isOutput=False)
    h["Wab"] = nc.declare_dram_parameter("Wab", [D, 1], F32, isOutput=False)
    h["bab"] = nc.declare_dram_parameter("bab", [1], F32, isOutput=False)
    h["Wd"] = nc.declare_dram_parameter("Wd", [D, 1], F32, isOutput=False)
    h["bd"] = nc.declare_dram_parameter("bd", [1], F32, isOutput=False)
    out_h = nc.declare_dram_parameter("out", [BL, T], F32, isOutput=True)
    dbg = {}
    if debug_taps:
        dbg["dbg_S"] = nc.declare_dram_parameter("dbg_S", [128, BL * (TS + 1)], F32, isOutput=True)
        dbg["dbg_reads"] = nc.declare_dram_parameter("dbg_reads", [128, NTOK], BF16, isOutput=True)

    with tile.TileContext(nc) as tc:
        _emit(nc, tc, h, out_h, dbg)
    nc.finalize()
    return nc


def _emit(nc, tc, h, out_h, dbg=None):
    from contextlib import ExitStack

    ctx = ExitStack()
    with ctx:
        persist = ctx.enter_context(tc.tile_pool(name="persist", bufs=1))
        dram = ctx.enter_context(tc.tile_pool(name="dram", bufs=1, space="DRAM"))

        # unnormalized softmax weights, [b, m, TS] bf16
        w_dram = dram.tile([BL, M, TS], BF16, name="w_dram")
        w_flat = w_dram.rearrange("b m t -> (b m t)")

        # persistent SBUF archives ([d, token] layouts, t-major tokens)
        k_T = persist.tile([128, NTOK], BF16)
        v_T = persist.tile([128, NTOK], BF16)
        e_T = persist.tile([128, NTOK], F32)
        erecip_T = persist.tile([128, NTOK], BF16)
        a_T = persist.tile([128, NTOK], F32)
        reads_T = persist.tile([128, NTOK], BF16)
        f_T = persist.tile([128, NTOK], BF16)

        ident = persist.tile([128, 128], F32)
        make_identity(nc, ident)

        We_f32 = persist.tile([128, 128], F32)
        nc.scalar.dma_start(out=We_f32, in_=h["We"][:, :])
        We_sb = persist.tile([128, 128], BF16)
        nc.scalar.copy(out=We_sb, in_=We_f32)
        Wa_f32 = persist.tile([128, 128], F32)
        nc.scalar.dma_start(out=Wa_f32, in_=h["Wa"][:, :])
        Wa_sb = persist.tile([128, 128], BF16)
        nc.scalar.copy(out=Wa_sb, in_=Wa_f32)
        Wf_r32 = persist.tile([128, 128], F32)
        nc.scalar.dma_start(out=Wf_r32, in_=h["Wf"][0:128, :])
        Wf_r = persist.tile([128, 128], BF16)
        nc.scalar.copy(out=Wf_r, in_=Wf_r32)
        Wf_k32 = persist.tile([128, 128], F32)
        nc.scalar.dma_start(out=Wf_k32, in_=h["Wf"][128:256, :])
        Wf_k = persist.tile([128, 128], BF16)
        nc.scalar.copy(out=Wf_k, in_=Wf_k32)
        Wab_sb = persist.tile([128, 1], F32)
        nc.sync.dma_start(out=Wab_sb, in_=h["Wab"][:, :])
        Wd_sb = persist.tile([128, 1], F32)
        nc.sync.dma_start(out=Wd_sb, in_=h["Wd"][:, :])
        Mk_sb = persist.tile([50, 128], F32)
        nc.sync.dma_start(out=Mk_sb, in_=h["Mk"][:, :])
        Mv0_sb = persist.tile([50, 128], F32)
        nc.sync.dma_start(out=Mv0_sb, in_=h["Mv0"][:, :])

        def col(name, n=128):
            t = persist.tile([n, 1], F32, name=name)
            nc.sync.dma_start(out=t, in_=_ap(h[name[:-4]][:], 0, [[1, n], [1, 1]]))
            return t

        be_col = col("be_col")
        ba_col = col("ba_col")
        bf_col = col("bf_col")

        # ---- transpose Mv0 and Mk once ----
        Mv0T_sb = persist.tile([128, 50], F32)
        MkT_sb = persist.tile([128, 50], BF16)
        S_init = persist.tile([128, 1], F32)
        with tc.tile_pool(name="init_ps", bufs=1, space="PSUM") as initp:
            mv0t = initp.tile([128, 50], F32)
            nc.tensor.transpose(mv0t, Mv0_sb, ident[0:50, 0:50])
            nc.scalar.copy(out=Mv0T_sb, in_=mv0t)
            mkt_ps = initp.tile([128, 50], F32)
            nc.tensor.transpose(mkt_ps, Mk_sb, ident[0:50, 0:50])
            nc.scalar.copy(out=MkT_sb, in_=mkt_ps)
        nc.vector.tensor_reduce(out=S_init, in_=Mv0T_sb,
                                axis=mybir.AxisListType.X, op=OP.add)

        # rhs staging: two persistent tiles, zero/ones written ONCE; the
        # diagonal scatter rewrites the same positions each group. The
        # scatter's AP makes the dep tracker attribute a byte range extending
        # ~one stage past the written region -> dead pad tile after each.
        rhs_stage = []
        for i in range(2):
            st = persist.tile([128, 8, NB, TB], BF16, name=f"rhsst{i}")
            pad = persist.tile([128, 4000], BF16, name=f"rhspad{i}")
            rhs_stage.append((st, pad))

        # lhsT tiles (persistent, rows 1..125 DMA-built per-b)
        lhs_g = [[persist.tile([128, 128], BF16, name=f"lhsg{b}_{i}") for i in range(NB)]
                 for b in range(BL)]

        # expw (unnormalized softmax numerators), [50m, b, 512t] bf16
        expwT = persist.tile([50, BL, T_PAD], BF16)

        # =========== phase 1: indices, on-chip table gathers ===========
        idxk_dram = dram.tile([NTOK], I16)
        idxv_dram = dram.tile([NTOK], I16)
        with tc.tile_pool(name="ph1", bufs=1) as ph1, \
             tc.tile_pool(name="ph1t", bufs=5) as ph1t, \
             tc.tile_pool(name="ph1ps", bufs=3, space="PSUM") as ph1ps:

            cseq = ph1.tile([8, T_PAD], I32)
            crse = ph1.tile([8, T_PAD], I32)
            nc.vector.memset(cseq, 0)
            nc.vector.memset(crse, 0)
            nc.sync.dma_start(out=cseq[:, 0:T], in_=h["concept_seq"][:, :])
            nc.scalar.dma_start(out=crse[:, 0:T], in_=h["correct_seq"][:, :])

            cseq_f = ph1.tile([8, T_PAD], F32)
            nc.vector.tensor_copy(out=cseq_f, in_=cseq)
            crse_f = ph1.tile([8, T_PAD], F32)
            nc.vector.tensor_copy(out=crse_f, in_=crse)
            x_f = ph1.tile([8, T_PAD], F32)
            nc.vector.scalar_tensor_tensor(out=x_f, in0=crse_f, scalar=float(NUM_CONCEPT),
                                           in1=cseq_f, op0=OP.mult, op1=OP.add)
            ck16s = ph1.tile([8, T_PAD], I16)
            nc.vector.tensor_copy(out=ck16s, in_=cseq)
            xv16s = ph1.tile([8, T_PAD], I16)
            nc.vector.tensor_copy(out=xv16s, in_=x_f)

            G16 = NTOK // 16
            ck16 = ph1.tile([128, G16], I16)
            xv16 = ph1.tile([128, G16], I16)
            for srct, drt, dstt, eng in ((ck16s, idxk_dram, ck16, nc.sync),
                                         (xv16s, idxv_dram, xv16, nc.scalar)):
                eng.dma_start(out=_ap(drt[:], 0, [[T_PAD, 8], [1, T_PAD]]), in_=srct)
                for j in range(8):
                    eng.dma_start(out=dstt[16 * j:16 * (j + 1), :],
                                  in_=_ap(drt[:], 0, [[1, 16], [16, G16]]))

            # tables transposed into [d, row] layout (f32: ap_gather needs
            # 4-byte elements)
            ekt = ph1.tile([128, NUM_CONCEPT], F32)
            evt = ph1.tile([128, 2 * NUM_CONCEPT], F32)
            gi = 0
            for tbl, dst_t, nrows in ((h["embed_key"], ekt, NUM_CONCEPT),
                                      (h["embed_value"], evt, 2 * NUM_CONCEPT)):
                for g0 in range(0, nrows, 512):
                    gn = min(512, nrows - g0)
                    nq = (gn + 127) // 128
                    rows4 = ph1t.tile([128, 512], F32, tag="rows4")
                    eng = (nc.sync, nc.gpsimd, nc.scalar)[gi % 3]
                    gi += 1
                    full = gn // 128
                    if full:
                        eng.dma_start(
                            out=rows4[:, 0:full * 128].rearrange("p (q c) -> p q c", c=128),
                            in_=_ap(tbl[:], g0 * 128,
                                    [[128, 128], [128 * 128, full], [1, 128]]))
                    if gn % 128:
                        rem = gn % 128
                        eng.dma_start(
                            out=rows4[0:rem, full * 128:(full + 1) * 128],
                            in_=tbl[g0 + full * 128:g0 + gn, :])
                    for q in range(nq):
                        n = min(128, gn - q * 128)
                        tps = ph1ps.tile([128, 128], F32, tag="tps")
                        nc.tensor.transpose(tps[:, 0:n],
                                            rows4[0:n, q * 128:(q + 1) * 128],
                                            ident[0:n, 0:n])
                        nc.scalar.copy(out=dst_t[:, g0 + q * 128:g0 + q * 128 + n],
                                       in_=tps[:, 0:n])

            gat = ph1.tile([128, NTOK], F32, name="gat")
            gatk = ph1.tile([128, NTOK], F32, name="gatk")
            HT = NTOK // 2
            for hf in range(2):
                hsl = slice(hf * HT, (hf + 1) * HT)
                isl = slice(hf * (HT // 16), (hf + 1) * (HT // 16))
                nc.gpsimd.ap_gather(
                    out_ap=gat[:, hsl].rearrange("p (n d) -> p n d", d=1),
                    in_ap=evt.rearrange("p (n d) -> p n d", d=1),
                    idxs_ap=xv16[:, isl], channels=128,
                    num_elems=2 * NUM_CONCEPT, d=1, num_idxs=HT)
                nc.vector.tensor_copy(out=v_T[:, hsl], in_=gat[:, hsl])
                nc.gpsimd.ap_gather(
                    out_ap=gatk[:, hsl].rearrange("p (n d) -> p n d", d=1),
                    in_ap=ekt.rearrange("p (n d) -> p n d", d=1),
                    idxs_ap=ck16[:, isl], channels=128,
                    num_elems=NUM_CONCEPT, d=1, num_idxs=HT)
                nc.gpsimd.tensor_copy(out=k_T[:, hsl], in_=gatk[:, hsl])

        # deferred staging init (keeps the Pool queue clear during phase 1)
        ones_src = persist.tile([1, 512], BF16, name="ones_src")
        nc.vector.memset(ones_src, 1.0)
        ones50 = persist.tile([50, 1], BF16, name="ones50")
        nc.vector.memset(ones50, 1.0)
        for st, pad in rhs_stage:
            nc.gpsimd.memset(st, 0.0)
            nc.sync.dma_start(out=st[0:1, :, :, :],
                              in_=_ap(ones_src[:, :], 0, [[1, 1], [0, 8 * NB], [1, TB]]))
            nc.vector.memset(pad[0:1, 0:1], 0.0)  # keep pad allocated
        rhs_stage = [st for st, _ in rhs_stage]
        for b in range(BL):
            for i in range(NB):
                nc.sync.dma_start(out=lhs_g[b][i][0:1, :],
                                  in_=ones_src[0:1, 0:128])

        # =========== phase 2 waves: e, a, 1/e, logits^T, expw ===========
        with tc.tile_pool(name="ph2ps", bufs=1, space="PSUM") as plps:
            def wave_e(cs):
                with nc.allow_low_precision(reason="bf16 1/e archive, 2e-2 tol"):
                    for c in cs:
                        sl = slice(c * 512, (c + 1) * 512)
                        elog = plps.tile([128, 512], F32, tag="elog", bufs=2)
                        nc.tensor.matmul(elog, We_sb, v_T[:, sl], start=True, stop=True)
                        nc.scalar.activation(out=e_T[:, sl], in_=elog, func=AF.Sigmoid, bias=be_col)
                        nc.vector.reciprocal(out=erecip_T[:, sl], in_=e_T[:, sl])
            def wave_a(cs):
                for c in cs:
                    sl = slice(c * 512, (c + 1) * 512)
                    alog = plps.tile([128, 512], F32, tag="elog", bufs=2)
                    nc.tensor.matmul(alog, Wa_sb, v_T[:, sl], start=True, stop=True)
                    nc.scalar.activation(out=a_T[:, sl], in_=alog, func=AF.Tanh, bias=ba_col)
            def wave_w(bs):
                for b in bs:
                    sl = slice(b * 512, (b + 1) * 512)
                    wlogT = plps.tile([50, 512], F32, tag="wlogT", bufs=2)
                    nc.tensor.matmul(wlogT, MkT_sb, k_T[:, sl], start=True, stop=True)
                    nc.scalar.activation(out=expwT[:, b, :], in_=wlogT, func=AF.Exp)
            wave_e([0]); wave_a([0]); wave_w([0])
            wave_e(range(1, 8)); wave_a(range(1, 8)); wave_w(range(1, 8))

        # =========== per-b: plumbing + scans + reads + heads ===========
        a_bt = a_T.rearrange("p (b t) -> p b t", t=T_PAD)
        er_bt = erecip_T.rearrange("p (b t) -> p b t", t=T_PAD)
        rd_bt = reads_T.rearrange("p (b t) -> p b t", t=T_PAD)
        nc.vector.memset(
            reads_T.rearrange("p (b t) -> p b t", t=T_PAD)[:, :, T:T_PAD], 0.0)

        rec = ctx.enter_context(tc.tile_pool(name="rec", bufs=2))
        rec1 = ctx.enter_context(tc.tile_pool(name="rec1", bufs=3))
        recps = ctx.enter_context(tc.tile_pool(name="recps", bufs=1, space="PSUM"))
        plps = ctx.enter_context(tc.tile_pool(name="plps", bufs=1, space="PSUM"))
        finps = ctx.enter_context(tc.tile_pool(name="finps", bufs=1, space="PSUM"))
        fin = ctx.enter_context(tc.tile_pool(name="fin", bufs=2))

        gpsP = [recps.tile([128, 2, 512], F32, name=f"gpsP{i}") for i in range(2)]
        gps2 = [recps.tile([128, 512], F32, name=f"gps{i}") for i in range(2)]

        # phase-4 head constants
        Wab0 = persist.tile([128, 2], BF16)
        nc.gpsimd.memset(Wab0, 0.0)
        nc.vector.tensor_copy(out=Wab0[:, 0:1], in_=Wab_sb)
        W0d = persist.tile([128, 2], BF16)
        nc.gpsimd.memset(W0d, 0.0)
        nc.vector.tensor_copy(out=W0d[:, 1:2], in_=Wd_sb)
        comb = persist.tile([2, 1], BF16)
        nc.gpsimd.memset(comb, -1.0)
        nc.gpsimd.memset(comb[0:1, :], 3.0)
        bias2 = persist.tile([2, 1], F32)
        nc.sync.dma_start(out=bias2[0:1, :], in_=_ap(h["bab"][:], 0, [[1, 1], [1, 1]]))
        nc.sync.dma_start(out=bias2[1:2, :], in_=_ap(h["bd"][:], 0, [[1, 1], [1, 1]]))

        gp = 0
        grp = 0
        for b in range(BL):
            sl = slice(b * 512, (b + 1) * 512)
            # ---- plumb(b) ----
            zps = plps.tile([1, 512], F32, tag="tp2", bufs=1)
            nc.tensor.matmul(zps, ones50, expwT[:, b, :], start=True, stop=True)
            zrow = rec1.tile([1, 512], F32, tag="zrow", bufs=1)
            nc.gpsimd.tensor_copy(out=zrow, in_=zps)
            ztile = plps.tile([TB, 264], F32, tag="tp2", bufs=1)
            zt_ps = ztile[:, 256:264]
            for i in range(NB):
                nc.tensor.transpose(zt_ps[:, i:i + 1], zrow[:, i * TB:(i + 1) * TB],
                                    ident[0:1, 0:1])
            rz = rec1.tile([TB, 8], F32, tag="rz")
            with nc.allow_low_precision(reason="1/Z softmax scale"):
                nc.vector.reciprocal(out=rz[:, 0:NB], in_=zt_ps[:, 0:NB])
            nc.vector.tensor_scalar(out=rz[:, 4:4 + NB], in0=rz[:, 0:NB],
                                    scalar1=-1.0, scalar2=None, op0=OP.mult)

            for i in range(NB):
                t0 = b * T_PAD + i * TB
                tp2 = plps.tile([TB, 264], F32, tag="tp2", bufs=1)
                nc.tensor.transpose(tp2[:, 0:128], e_T[:, t0:t0 + TB], ident)
                ea2 = rec1.tile([TB, 128], BF16, tag="ea2")
                nc.scalar.activation(out=ea2, in_=tp2[:, 0:128],
                                     func=AF.Copy, scale=rz[:, 4 + i:5 + i])
                nc.sync.dma_start(out=lhs_g[b][i][1:1 + TB, :], in_=ea2)
            nc.sync.dma_start(out=_ap(w_flat[:], b * M * TS, [[TS, 50], [1, TS]]),
                              in_=expwT[:, b, 0:TS])

            # car = a/e (c shifted), D_t = car_t - car_{t+1}; y-scan init tile
            car = rec1.tile([128, T_PAD], F32, tag="car", bufs=2)
            nc.gpsimd.tensor_tensor(out=car[:, 0:T], in0=a_bt[:, b, 0:T],
                                    in1=er_bt[:, b, 0:T], op=OP.mult)
            nc.gpsimd.memset(car[:, T:T + 1], 0.0)
            Dt = rec1.tile([128, T], F32, tag="Dt", bufs=2)
            nc.gpsimd.tensor_tensor(out=Dt, in0=car[:, 0:T], in1=car[:, 1:T + 1],
                                    op=OP.subtract)
            D50 = rec1.tile([128, T], F32, tag="D50", bufs=1)
            nc.scalar.activation(out=D50, in_=Dt, func=AF.Copy, scale=float(M))
            initb = rec1.tile([128, 50], F32, tag="initb", bufs=2)
            nc.vector.tensor_scalar(out=initb, in0=Mv0T_sb, scalar1=car[:, 0:1],
                                    scalar2=None, op0=OP.subtract)

            # ---- phase 3(b) ----
            S_b = rec.tile([128, TS + 1], F32, tag="sb")
            nc.vector.scalar_tensor_tensor(out=S_b[:, 0:1], in0=car[:, 0:1],
                                           scalar=-float(M), in1=S_init,
                                           op0=OP.mult, op1=OP.add)
            for mc0 in range(0, M, 8):
                msz = min(8, M - mc0)
                rhs = rhs_stage[grp % 2]
                grp += 1
                rap = rhs[:, :, :, :]
                pstr = rap.ap[0][0]
                dst = bass.AP(rap.tensor, rap.offset + pstr,
                              [[pstr + 1, TB], [NB * TB, msz], [TB, NB]])
                src = _ap(w_flat[:], (b * M + mc0) * TS,
                          [[1, TB], [TS, msz], [TB, NB]])
                nc.sync.dma_start(out=dst, in_=src)

                traj = rec.tile([128, 8, TS], F32, tag="traj")
                ndve = 2
                for mi in range(ndve):
                    m = mc0 + mi
                    gt = gps2[gp % 2]
                    for i in range(NB):
                        nc.tensor.matmul(gt[:, i * TB:(i + 1) * TB],
                                         lhs_g[b][i][0:1 + TB, :],
                                         rhs[0:1 + TB, mi, i, :],
                                         start=True, stop=True)
                    nc.vector.tensor_tensor_scan(
                        out=traj[:, mi, :],
                        data0=gt[:, 0:TS],
                        data1=Dt,
                        initial=initb[:, m:m + 1],
                        op0=OP.mult, op1=OP.add)
                    gp += 1
                # SBUF-fed scans: g pairs evac'd by ACT (dodges the per-scan
                # PSUM access penalty on DVE)
                pg = 0
                for p0 in range(ndve, msz, 2):
                    psz = min(2, msz - p0)
                    gq = gpsP[pg % 2]
                    pg += 1
                    for mi in range(p0, p0 + psz):
                        for i in range(NB):
                            nc.tensor.matmul(gq[:, mi - p0, i * TB:(i + 1) * TB],
                                             lhs_g[b][i][0:1 + TB, :],
                                             rhs[0:1 + TB, mi, i, :],
                                             start=True, stop=True)
                    gsb = rec1.tile([128, 2, TS], F32, tag="gsb")
                    nc.scalar.copy(out=gsb[:, 0:psz, :], in_=gq[:, 0:psz, 0:TS])
                    for mi in range(p0, p0 + psz):
                        m = mc0 + mi
                        nc.vector.tensor_tensor_scan(
                            out=traj[:, mi, :],
                            data0=gsb[:, mi - p0, :],
                            data1=Dt,
                            initial=initb[:, m:m + 1],
                            op0=OP.mult, op1=OP.add)

                # reduce over the group's m via TT-add tree -> S
                tap = traj[:, :, :]
                part = list(tap.ap[0])
                t4 = rec.tile([128, 4, TS], F32, tag="t4", bufs=2)
                if msz == 8:
                    nc.gpsimd.tensor_tensor(
                        out=t4,
                        in0=_ap(tap, tap.offset, [part, [2 * TS, 4], [1, TS]]),
                        in1=_ap(tap, tap.offset + TS, [part, [2 * TS, 4], [1, TS]]),
                        op=OP.add)
                    nc.gpsimd.tensor_tensor(
                        out=t4[:, 0:2, :], in0=t4[:, 0:2, :], in1=t4[:, 2:4, :],
                        op=OP.add)
                    spart = t4[:, 0, :]
                    nc.gpsimd.tensor_tensor(out=spart, in0=spart,
                                            in1=t4[:, 1, :], op=OP.add)
                else:  # msz == 2
                    spart = t4[:, 0, :]
                    nc.gpsimd.tensor_tensor(out=spart, in0=traj[:, 0, :],
                                            in1=traj[:, 1, :], op=OP.add)
                if mc0 == 0:
                    nc.gpsimd.tensor_copy(out=S_b[:, 1:TS + 1], in_=spart)
                else:
                    nc.gpsimd.tensor_tensor(out=S_b[:, 1:TS + 1],
                                            in0=S_b[:, 1:TS + 1],
                                            in1=spart, op=OP.add)

            # ---- reads(b) on Pool ----
            d1 = rec1.tile([128, T], F32, tag="d1", bufs=1)
            nc.gpsimd.tensor_tensor(out=d1, in0=S_b[:, 0:T],
                                    in1=S_b[:, 1:T + 1], op=OP.subtract)
            nc.gpsimd.tensor_tensor(out=d1, in0=d1, in1=D50, op=OP.add)
            nc.gpsimd.tensor_tensor(out=d1, in0=d1, in1=a_bt[:, b, 0:T], op=OP.add)
            nc.gpsimd.tensor_tensor(out=rd_bt[:, b, 0:T], in0=d1,
                                    in1=er_bt[:, b, 0:T], op=OP.mult)

            # ---- phase 4(b) ----
            f_ps = finps.tile([128, 512], F32, tag="fps")
            nc.tensor.matmul(f_ps, Wf_r, reads_T[:, sl], start=True, stop=False)
            nc.tensor.matmul(f_ps, Wf_k, k_T[:, sl], start=False, stop=True)
            nc.scalar.activation(out=f_T[:, sl], in_=f_ps, func=AF.Tanh, bias=bf_col)
            hp2 = finps.tile([2, 512], F32, tag="fps")
            nc.tensor.matmul(hp2, Wab0, f_T[:, sl], start=True, stop=False)
            nc.tensor.matmul(hp2, W0d, k_T[:, sl], start=False, stop=True)
            ht = fin.tile([2, 512], BF16, tag="ht", bufs=1)
            nc.scalar.activation(out=ht, in_=hp2, func=AF.Tanh, bias=bias2)
            lg_ps = finps.tile([1, 512], F32, tag="fps")
            nc.tensor.matmul(lg_ps, comb, ht, start=True, stop=True)
            prob_row = fin.tile([1, 512], F32, tag="prob", bufs=1)
            nc.scalar.activation(out=prob_row, in_=lg_ps, func=AF.Sigmoid)
            nc.sync.dma_start(out=out_h[b:b + 1, :], in_=prob_row[0:1, 0:T])

        if dbg:
            nc.sync.dma_start(out=dbg["dbg_reads"][:, :], in_=reads_T)


_NC = None
LAST_RESULT = None


def _get_nc():
    global _NC
    if _NC is None:
        _NC = build_program()
    return _NC


def kernel(**inputs):
    global LAST_RESULT
    from concourse.bass_utils import run_bass_kernel_spmd

    nc = _get_nc()
    names = ["concept_seq", "correct_seq", "embed_key", "embed_value", "Mk", "Mv0",
             "We", "be", "Wa", "ba", "Wf", "bf", "Wab", "bab", "Wd", "bd"]
    full = {k: np.ascontiguousarray(np.asarray(inputs[k])) for k in names}
    in_maps = []
    for i in range(NCORES):
        m = dict(full)
        m["concept_seq"] = np.ascontiguousarray(full["concept_seq"][i * BL:(i + 1) * BL])
        m["correct_seq"] = np.ascontiguousarray(full["correct_seq"][i * BL:(i + 1) * BL])
        in_maps.append(m)
    res = run_bass_kernel_spmd(nc, in_maps, core_ids=list(range(NCORES)))
    LAST_RESULT = res
    return np.concatenate([res.results[i]["out"] for i in range(NCORES)], axis=0)


if __name__ == "__main__":
    nc = build_program()
    print("build ok")


# revision 7
# speedup vs baseline: 1.0148x; 1.0028x over previous
"""DeepIRT (DKVMN) Trainium2 kernel — scan architecture, per-batch pipelined.

Contract: kernel(**inputs) takes the FULL unsharded inputs of reference.py's
setup_inputs() and returns the full [64, 500] float32 output.

Strategy (8 NeuronCores, pure data parallel over batch; BL=8 rows per core):
  phase 1: gather k/v embeddings into [128d, tok] bf16 SBUF archives
    (tok = b*512 + t; tables pre-transposed/cast to bf16, gathered directly).
  phase 2 (up-front waves, grouped by ACT function to avoid table reloads):
    e = sigmoid(vWe+be) (f32), a = tanh(vWa+ba) (f32), 1/e (bf16);
    logits^T = Mk k^T via one [50m, 512t] matmul per b; expw = exp (bf16).
  per-b pipeline (b = 0..7), emitted so phase3(b) overlaps plumbing(b+1):
    plumb(b):
      Z = sum_m expw (Pool partition-reduce), rz = 1/Z^T ([125t, 4blk] via PE)
      e/a transposed per 125-block via PE; ACT-evac'd bf16 with scale -+rz
        (folds softmax normalization into the coefficients):
        lhs_g[b][blk] = [ones_row; -e^T/Z rows]   [126, 128]
        lhs_a[b][blk] = [zeros_row; a^T/Z rows]
        (rows 1..125 DMA'd in with +1 partition shift)
      expw written to w_dram [b, m, 500] bf16
    phase 3(b): for each m (2-m groups):
      rhs = [ones_row; diag(expw[b,m,blk])] via diagonal-scatter DMAs into
        persistent zeroed staging tiles (only diag positions rewritten)
      g = lhs_g^T @ rhs = 1 - w*e -> PSUM f32 [128, 500]    (4 matmuls)
      h = lhs_a^T @ rhs = w*a     -> PSUM, ACT-evac'd bf16  (4 matmuls)
      traj[m] = tensor_tensor_scan(data0=g[PSUM], data1=h, init=Mv0T[:,m])
        on DVE: one instruction runs all 500 steps, fp32 internal state
      Pool TT-tree reduces traj over m into S[b][d, 1+t] = sum_m Mv_t
    reads(b) on Pool: read_t = (S[t-1] - S[t] + a_t) * (1/e_t)
    phase 4(b): f = tanh([reads, k] Wf + bf); heads; sigmoid(3*stu - qd).
"""

import numpy as np

import concourse.bass as bass
import concourse.bacc as bacc
import concourse.tile as tile
import concourse.mybir as mybir
from concourse.masks import make_identity

F32 = mybir.dt.float32
BF16 = mybir.dt.bfloat16
I32 = mybir.dt.int32
I16 = mybir.dt.int16
OP = mybir.AluOpType
AF = mybir.ActivationFunctionType

NUM_CONCEPT = 1000
D = 128
M = 50
B_FULL, T = 64, 500
NCORES = 8
BL = B_FULL // NCORES          # 8 batch rows per core
T_PAD = 512
NTOK = T_PAD * BL              # 4096 padded tokens, b-major: tok = b*T_PAD + t
TB = 125                       # scan time block
NB = 4                         # blocks per scan
TS = TB * NB                   # 500 = T exactly


def _ap(t, offset, dims):
    return bass.AP(t.tensor, offset, [list(d) for d in dims])


def build_program(debug_taps=False):
    nc = bacc.Bacc("TRN2", target_bir_lowering=False, debug=False)

    h = {}
    h["concept_seq"] = nc.declare_dram_parameter("concept_seq", [BL, T], I32, isOutput=False)
    h["correct_seq"] = nc.declare_dram_parameter("correct_seq", [BL, T], I32, isOutput=False)
    h["embed_key"] = nc.declare_dram_parameter("embed_key", [NUM_CONCEPT, D], F32, isOutput=False)
    h["embed_value"] = nc.declare_dram_parameter("embed_value", [2 * NUM_CONCEPT, D], F32, isOutput=False)
    h["Mk"] = nc.declare_dram_parameter("Mk", [M, D], F32, isOutput=False)
    h["Mv0"] = nc.declare_dram_parameter("Mv0", [M, D], F32, isOutput=False)
    h["We"] = nc.declare_dram_parameter("We", [D, D], F32, isOutput=False)
    h["be"] = nc.declare_dram_parameter("be", [D], F32, isOutput=False)
    h["Wa"] = nc.declare_dram_parameter("Wa", [D, D], F32, isOutput=False)
    h["ba"] = nc.declare_dram_parameter("ba", [D], F32, isOutput=False)
    h["Wf"] = nc.declare_dram_parameter("Wf", [2 * D, D], F32, isOutput=False)
    h["bf"] = nc.declare_dram_parameter("bf", [D], F32, isOutput=False)
    h["Wab"] = nc.declare_dram_parameter("Wab", [D, 1], F32, isOutput=False)
    h["bab"] = nc.declare_dram_parameter("bab", [1], F32, isOutput=False)
    h["Wd"] = nc.declare_dram_parameter("Wd", [D, 1], F32, isOutput=False)
    h["bd"] = nc.declare_dram_parameter("bd", [1], F32, isOutput=False)
    out_h = nc.declare_dram_parameter("out", [BL, T], F32, isOutput=True)
    dbg = {}
    if debug_taps:
        dbg["dbg_S"] = nc.declare_dram_parameter("dbg_S", [128, BL * (TS + 1)], F32, isOutput=True)
        dbg["dbg_reads"] = nc.declare_dram_parameter("dbg_reads", [128, NTOK], BF16, isOutput=True)

    with tile.TileContext(nc) as tc:
        _emit(nc, tc, h, out_h, dbg)
    nc.finalize()
    return nc


def _emit(nc, tc, h, out_h, dbg=None):
    from contextlib import ExitStack

    ctx = ExitStack()
    with ctx:
        persist = ctx.enter_context(tc.tile_pool(name="persist", bufs=1))
        dram = ctx.enter_context(tc.tile_pool(name="dram", bufs=1, space="DRAM"))

        # unnormalized softmax weights, [b, m, TS] bf16
        w_dram = dram.tile([BL, M, TS], BF16, name="w_dram")
        w_flat = w_dram.rearrange("b m t -> (b m t)")

        # persistent SBUF archives ([d, token] layouts, t-major tokens)
        k_T = persist.tile([128, NTOK], BF16)
        v_T = persist.tile([128, NTOK], BF16)
        e_T = persist.tile([128, NTOK], F32)
        erecip_T = persist.tile([128, NTOK], BF16)
        a_T = persist.tile([128, NTOK], F32)
        reads_T = persist.tile([128, NTOK], BF16)
        f_T = persist.tile([128, NTOK], BF16)

        ident = persist.tile([128, 128], F32)
        make_identity(nc, ident)

        We_f32 = persist.tile([128, 128], F32)
        nc.scalar.dma_start(out=We_f32, in_=h["We"][:, :])
        We_sb = persist.tile([128, 128], BF16)
        nc.scalar.copy(out=We_sb, in_=We_f32)
        Wa_f32 = persist.tile([128, 128], F32)
        nc.scalar.dma_start(out=Wa_f32, in_=h["Wa"][:, :])
        Wa_sb = persist.tile([128, 128], BF16)
        nc.scalar.copy(out=Wa_sb, in_=Wa_f32)
        Wf_r32 = persist.tile([128, 128], F32)
        nc.scalar.dma_start(out=Wf_r32, in_=h["Wf"][0:128, :])
        Wf_r = persist.tile([128, 128], BF16)
        nc.scalar.copy(out=Wf_r, in_=Wf_r32)
        Wf_k32 = persist.tile([128, 128], F32)
        nc.scalar.dma_start(out=Wf_k32, in_=h["Wf"][128:256, :])
        Wf_k = persist.tile([128, 128], BF16)
        nc.scalar.copy(out=Wf_k, in_=Wf_k32)
        Wab_sb = persist.tile([128, 1], F32)
        nc.sync.dma_start(out=Wab_sb, in_=h["Wab"][:, :])
        Wd_sb = persist.tile([128, 1], F32)
        nc.sync.dma_start(out=Wd_sb, in_=h["Wd"][:, :])
        Mk_sb = persist.tile([50, 128], F32)
        nc.sync.dma_start(out=Mk_sb, in_=h["Mk"][:, :])
        Mv0_sb = persist.tile([50, 128], F32)
        nc.sync.dma_start(out=Mv0_sb, in_=h["Mv0"][:, :])

        def col(name, n=128):
            t = persist.tile([n, 1], F32, name=name)
            nc.sync.dma_start(out=t, in_=_ap(h[name[:-4]][:], 0, [[1, n], [1, 1]]))
            return t

        be_col = col("be_col")
        ba_col = col("ba_col")
        bf_col = col("bf_col")

        # ---- transpose Mv0 and Mk once ----
        Mv0T_sb = persist.tile([128, 50], F32)
        MkT_sb = persist.tile([128, 50], BF16)
        S_init = persist.tile([128, 1], F32)
        with tc.tile_pool(name="init_ps", bufs=1, space="PSUM") as initp:
            mv0t = initp.tile([128, 50], F32)
            nc.tensor.transpose(mv0t, Mv0_sb, ident[0:50, 0:50])
            nc.scalar.copy(out=Mv0T_sb, in_=mv0t)
            mkt_ps = initp.tile([128, 50], F32)
            nc.tensor.transpose(mkt_ps, Mk_sb, ident[0:50, 0:50])
            nc.scalar.copy(out=MkT_sb, in_=mkt_ps)
        nc.vector.tensor_reduce(out=S_init, in_=Mv0T_sb,
                                axis=mybir.AxisListType.X, op=OP.add)

        # rhs staging: two persistent tiles, zero/ones written ONCE; the
        # diagonal scatter rewrites the same positions each group. The
        # scatter's AP makes the dep tracker attribute a byte range extending
        # ~one stage past the written region -> dead pad tile after each.
        rhs_stage = []
        for i in range(2):
            st = persist.tile([128, 8, NB, TB], BF16, name=f"rhsst{i}")
            pad = persist.tile([128, 4000], BF16, name=f"rhspad{i}")
            rhs_stage.append((st, pad))

        # lhsT tiles (persistent, rows 1..125 DMA-built per-b)
        lhs_g = [[persist.tile([128, 128], BF16, name=f"lhsg{b}_{i}") for i in range(NB)]
                 for b in range(BL)]

        # expw (unnormalized softmax numerators), [50m, b, 512t] bf16
        expwT = persist.tile([50, BL, T_PAD], BF16)

        # =========== phase 1: indices, on-chip table gathers ===========
        idxk_dram = dram.tile([NTOK], I16)
        idxv_dram = dram.tile([NTOK], I16)
        with tc.tile_pool(name="ph1", bufs=1) as ph1, \
             tc.tile_pool(name="ph1t", bufs=5) as ph1t, \
             tc.tile_pool(name="ph1ps", bufs=3, space="PSUM") as ph1ps:

            cseq = ph1.tile([8, T_PAD], I32)
            crse = ph1.tile([8, T_PAD], I32)
            nc.vector.memset(cseq, 0)
            nc.vector.memset(crse, 0)
            nc.sync.dma_start(out=cseq[:, 0:T], in_=h["concept_seq"][:, :])
            nc.scalar.dma_start(out=crse[:, 0:T], in_=h["correct_seq"][:, :])

            cseq_f = ph1.tile([8, T_PAD], F32)
            nc.vector.tensor_copy(out=cseq_f, in_=cseq)
            crse_f = ph1.tile([8, T_PAD], F32)
            nc.vector.tensor_copy(out=crse_f, in_=crse)
            x_f = ph1.tile([8, T_PAD], F32)
            nc.vector.scalar_tensor_tensor(out=x_f, in0=crse_f, scalar=float(NUM_CONCEPT),
                                           in1=cseq_f, op0=OP.mult, op1=OP.add)
            ck16s = ph1.tile([8, T_PAD], I16)
            nc.vector.tensor_copy(out=ck16s, in_=cseq)
            xv16s = ph1.tile([8, T_PAD], I16)
            nc.vector.tensor_copy(out=xv16s, in_=x_f)

            G16 = NTOK // 16
            ck16 = ph1.tile([128, G16], I16)
            xv16 = ph1.tile([128, G16], I16)
            for srct, drt, dstt, eng in ((ck16s, idxk_dram, ck16, nc.sync),
                                         (xv16s, idxv_dram, xv16, nc.scalar)):
                eng.dma_start(out=_ap(drt[:], 0, [[T_PAD, 8], [1, T_PAD]]), in_=srct)
                for j in range(8):
                    eng.dma_start(out=dstt[16 * j:16 * (j + 1), :],
                                  in_=_ap(drt[:], 0, [[1, 16], [16, G16]]))

            # tables transposed into [d, row] layout (f32: ap_gather needs
            # 4-byte elements)
            ekt = ph1.tile([128, NUM_CONCEPT], F32)
            evt = ph1.tile([128, 2 * NUM_CONCEPT], F32)
            gi = 0
            for tbl, dst_t, nrows in ((h["embed_key"], ekt, NUM_CONCEPT),
                                      (h["embed_value"], evt, 2 * NUM_CONCEPT)):
                for g0 in range(0, nrows, 512):
                    gn = min(512, nrows - g0)
                    nq = (gn + 127) // 128
                    rows4 = ph1t.tile([128, 512], F32, tag="rows4")
                    eng = (nc.sync, nc.gpsimd, nc.scalar)[gi % 3]
                    gi += 1
                    full = gn // 128
                    if full:
                        eng.dma_start(
                            out=rows4[:, 0:full * 128].rearrange("p (q c) -> p q c", c=128),
                            in_=_ap(tbl[:], g0 * 128,
                                    [[128, 128], [128 * 128, full], [1, 128]]))
                    if gn % 128:
                        rem = gn % 128
                        eng.dma_start(
                            out=rows4[0:rem, full * 128:(full + 1) * 128],
                            in_=tbl[g0 + full * 128:g0 + gn, :])
                    for q in range(nq):
                        n = min(128, gn - q * 128)
                        tps = ph1ps.tile([128, 128], F32, tag="tps")
                        nc.tensor.transpose(tps[:, 0:n],
                                            rows4[0:n, q * 128:(q + 1) * 128],
                                            ident[0:n, 0:n])
                        nc.scalar.copy(out=dst_t[:, g0 + q * 128:g0 + q * 128 + n],
                                       in_=tps[:, 0:n])

            gat = ph1.tile([128, NTOK], F32, name="gat")
            gatk = ph1.tile([128, NTOK], F32, name="gatk")
            HT = NTOK // 2
            for hf in range(2):
                hsl = slice(hf * HT, (hf + 1) * HT)
                isl = slice(hf * (HT // 16), (hf + 1) * (HT // 16))
                nc.gpsimd.ap_gather(
                    out_ap=gat[:, hsl].rearrange("p (n d) -> p n d", d=1),
                    in_ap=evt.rearrange("p (n d) -> p n d", d=1),
                    idxs_ap=xv16[:, isl], channels=128,
                    num_elems=2 * NUM_CONCEPT, d=1, num_idxs=HT)
                nc.vector.tensor_copy(out=v_T[:, hsl], in_=gat[:, hsl])
                nc.gpsimd.ap_gather(
                    out_ap=gatk[:, hsl].rearrange("p (n d) -> p n d", d=1),
                    in_ap=ekt.rearrange("p (n d) -> p n d", d=1),
                    idxs_ap=ck16[:, isl], channels=128,
                    num_elems=NUM_CONCEPT, d=1, num_idxs=HT)
                nc.gpsimd.tensor_copy(out=k_T[:, hsl], in_=gatk[:, hsl])

        # deferred staging init (keeps the Pool queue clear during phase 1)
        ones_src = persist.tile([1, 512], BF16, name="ones_src")
        nc.vector.memset(ones_src, 1.0)
        ones50 = persist.tile([50, 1], BF16, name="ones50")
        nc.vector.memset(ones50, 1.0)
        for st, pad in rhs_stage:
            nc.gpsimd.memset(st, 0.0)
            nc.sync.dma_start(out=st[0:1, :, :, :],
                              in_=_ap(ones_src[:, :], 0, [[1, 1], [0, 8 * NB], [1, TB]]))
            nc.vector.memset(pad[0:1, 0:1], 0.0)  # keep pad allocated
        rhs_stage = [st for st, _ in rhs_stage]
        for b in range(BL):
            for i in range(NB):
                nc.sync.dma_start(out=lhs_g[b][i][0:1, :],
                                  in_=ones_src[0:1, 0:128])

        # =========== phase 2 waves: e, a, 1/e, logits^T, expw ===========
        with tc.tile_pool(name="ph2ps", bufs=1, space="PSUM") as plps:
            def wave_e(cs):
                with nc.allow_low_precision(reason="bf16 1/e archive, 2e-2 tol"):
                    for c in cs:
                        sl = slice(c * 512, (c + 1) * 512)
                        elog = plps.tile([128, 512], F32, tag="elog", bufs=2)
                        nc.tensor.matmul(elog, We_sb, v_T[:, sl], start=True, stop=True)
                        nc.scalar.activation(out=e_T[:, sl], in_=elog, func=AF.Sigmoid, bias=be_col)
                        nc.vector.reciprocal(out=erecip_T[:, sl], in_=e_T[:, sl])
            def wave_a(cs):
                for c in cs:
                    sl = slice(c * 512, (c + 1) * 512)
                    alog = plps.tile([128, 512], F32, tag="elog", bufs=2)
                    nc.tensor.matmul(alog, Wa_sb, v_T[:, sl], start=True, stop=True)
                    nc.scalar.activation(out=a_T[:, sl], in_=alog, func=AF.Tanh, bias=ba_col)
            def wave_w(bs):
                for b in bs:
                    sl = slice(b * 512, (b + 1) * 512)
                    wlogT = plps.tile([50, 512], F32, tag="wlogT", bufs=2)
                    nc.tensor.matmul(wlogT, MkT_sb, k_T[:, sl], start=True, stop=True)
                    nc.scalar.activation(out=expwT[:, b, :], in_=wlogT, func=AF.Exp)
            wave_e([0]); wave_a([0]); wave_w([0])
            wave_e(range(1, 8)); wave_a(range(1, 8)); wave_w(range(1, 8))

        # =========== per-b: plumbing + scans + reads + heads ===========
        a_bt = a_T.rearrange("p (b t) -> p b t", t=T_PAD)
        er_bt = erecip_T.rearrange("p (b t) -> p b t", t=T_PAD)
        rd_bt = reads_T.rearrange("p (b t) -> p b t", t=T_PAD)
        nc.vector.memset(
            reads_T.rearrange("p (b t) -> p b t", t=T_PAD)[:, :, T:T_PAD], 0.0)

        rec = ctx.enter_context(tc.tile_pool(name="rec", bufs=2))
        rec1 = ctx.enter_context(tc.tile_pool(name="rec1", bufs=3))
        recps = ctx.enter_context(tc.tile_pool(name="recps", bufs=1, space="PSUM"))
        plps = ctx.enter_context(tc.tile_pool(name="plps", bufs=1, space="PSUM"))
        finps = ctx.enter_context(tc.tile_pool(name="finps", bufs=1, space="PSUM"))
        fin = ctx.enter_context(tc.tile_pool(name="fin", bufs=2))

        gpsP = [recps.tile([128, 2, 512], F32, name=f"gpsP{i}") for i in range(2)]
        gps2 = [recps.tile([128, 512], F32, name=f"gps{i}") for i in range(2)]

        # phase-4 head constants
        Wab0 = persist.tile([128, 2], BF16)
        nc.gpsimd.memset(Wab0, 0.0)
        nc.vector.tensor_copy(out=Wab0[:, 0:1], in_=Wab_sb)
        W0d = persist.tile([128, 2], BF16)
        nc.gpsimd.memset(W0d, 0.0)
        nc.vector.tensor_copy(out=W0d[:, 1:2], in_=Wd_sb)
        comb = persist.tile([2, 1], BF16)
        nc.gpsimd.memset(comb, -1.0)
        nc.gpsimd.memset(comb[0:1, :], 3.0)
        bias2 = persist.tile([2, 1], F32)
        nc.sync.dma_start(out=bias2[0:1, :], in_=_ap(h["bab"][:], 0, [[1, 1], [1, 1]]))
        nc.sync.dma_start(out=bias2[1:2, :], in_=_ap(h["bd"][:], 0, [[1, 1], [1, 1]]))

        gp = 0
        grp = 0
        for b in range(BL):
            sl = slice(b * 512, (b + 1) * 512)
            # ---- plumb(b) ----
            zps = plps.tile([1, 512], F32, tag="tp2", bufs=1)
            nc.tensor.matmul(zps, ones50, expwT[:, b, :], start=True, stop=True)
            zrow = rec1.tile([1, 512], F32, tag="zrow", bufs=1)
            nc.vector.tensor_copy(out=zrow, in_=zps)
            ztile = plps.tile([TB, 264], F32, tag="tp2", bufs=1)
            zt_ps = ztile[:, 256:264]
            for i in range(NB):
                nc.tensor.transpose(zt_ps[:, i:i + 1], zrow[:, i * TB:(i + 1) * TB],
                                    ident[0:1, 0:1])
            rz = rec1.tile([TB, 8], F32, tag="rz")
            with nc.allow_low_precision(reason="1/Z softmax scale"):
                nc.vector.reciprocal(out=rz[:, 0:NB], in_=zt_ps[:, 0:NB])
            nc.vector.tensor_scalar(out=rz[:, 4:4 + NB], in0=rz[:, 0:NB],
                                    scalar1=-1.0, scalar2=None, op0=OP.mult)

            for i in range(NB):
                t0 = b * T_PAD + i * TB
                tp2 = plps.tile([TB, 264], F32, tag="tp2", bufs=1)
                nc.tensor.transpose(tp2[:, 0:128], e_T[:, t0:t0 + TB], ident)
                ea2 = rec1.tile([TB, 128], BF16, tag="ea2")
                nc.scalar.activation(out=ea2, in_=tp2[:, 0:128],
                                     func=AF.Copy, scale=rz[:, 4 + i:5 + i])
                nc.sync.dma_start(out=lhs_g[b][i][1:1 + TB, :], in_=ea2)
            nc.sync.dma_start(out=_ap(w_flat[:], b * M * TS, [[TS, 50], [1, TS]]),
                              in_=expwT[:, b, 0:TS])

            # car = a/e (c shifted), D_t = car_t - car_{t+1}; y-scan init tile
            car = rec1.tile([128, T_PAD], F32, tag="car", bufs=2)
            nc.gpsimd.tensor_tensor(out=car[:, 0:T], in0=a_bt[:, b, 0:T],
                                    in1=er_bt[:, b, 0:T], op=OP.mult)
            nc.gpsimd.memset(car[:, T:T + 1], 0.0)
            Dt = rec1.tile([128, T], F32, tag="Dt", bufs=2)
            nc.gpsimd.tensor_tensor(out=Dt, in0=car[:, 0:T], in1=car[:, 1:T + 1],
                                    op=OP.subtract)
            D50 = rec1.tile([128, T], F32, tag="D50", bufs=1)
            nc.scalar.activation(out=D50, in_=Dt, func=AF.Copy, scale=float(M))
            initb = rec1.tile([128, 50], F32, tag="initb", bufs=2)
            nc.vector.tensor_scalar(out=initb, in0=Mv0T_sb, scalar1=car[:, 0:1],
                                    scalar2=None, op0=OP.subtract)

            # ---- phase 3(b) ----
            S_b = rec.tile([128, TS + 1], F32, tag="sb")
            nc.vector.scalar_tensor_tensor(out=S_b[:, 0:1], in0=car[:, 0:1],
                                           scalar=-float(M), in1=S_init,
                                           op0=OP.mult, op1=OP.add)
            for mc0 in range(0, M, 8):
                msz = min(8, M - mc0)
                rhs = rhs_stage[grp % 2]
                grp += 1
                rap = rhs[:, :, :, :]
                pstr = rap.ap[0][0]
                dst = bass.AP(rap.tensor, rap.offset + pstr,
                              [[pstr + 1, TB], [NB * TB, msz], [TB, NB]])
                src = _ap(w_flat[:], (b * M + mc0) * TS,
                          [[1, TB], [TS, msz], [TB, NB]])
                nc.sync.dma_start(out=dst, in_=src)

                traj = rec.tile([128, 8, TS], F32, tag="traj")
                ndve = 2
                for mi in range(ndve):
                    m = mc0 + mi
                    gt = gps2[gp % 2]
                    for i in range(NB):
                        nc.tensor.matmul(gt[:, i * TB:(i + 1) * TB],
                                         lhs_g[b][i][0:1 + TB, :],
                                         rhs[0:1 + TB, mi, i, :],
                                         start=True, stop=True)
                    nc.vector.tensor_tensor_scan(
                        out=traj[:, mi, :],
                        data0=gt[:, 0:TS],
                        data1=Dt,
                        initial=initb[:, m:m + 1],
                        op0=OP.mult, op1=OP.add)
                    gp += 1
                # SBUF-fed scans: g pairs evac'd by ACT (dodges the per-scan
                # PSUM access penalty on DVE)
                pg = 0
                for p0 in range(ndve, msz, 2):
                    psz = min(2, msz - p0)
                    gq = gpsP[pg % 2]
                    pg += 1
                    for mi in range(p0, p0 + psz):
                        for i in range(NB):
                            nc.tensor.matmul(gq[:, mi - p0, i * TB:(i + 1) * TB],
                                             lhs_g[b][i][0:1 + TB, :],
                                             rhs[0:1 + TB, mi, i, :],
                                             start=True, stop=True)
                    gsb = rec1.tile([128, 2, TS], F32, tag="gsb")
                    nc.scalar.copy(out=gsb[:, 0:psz, :], in_=gq[:, 0:psz, 0:TS])
                    for mi in range(p0, p0 + psz):
                        m = mc0 + mi
                        nc.vector.tensor_tensor_scan(
                            out=traj[:, mi, :],
                            data0=gsb[:, mi - p0, :],
                            data1=Dt,
                            initial=initb[:, m:m + 1],
                            op0=OP.mult, op1=OP.add)

                # reduce over the group's m via TT-add tree -> S
                tap = traj[:, :, :]
                part = list(tap.ap[0])
                t4 = rec.tile([128, 4, TS], F32, tag="t4", bufs=2)
                if msz == 8:
                    nc.gpsimd.tensor_tensor(
                        out=t4,
                        in0=_ap(tap, tap.offset, [part, [2 * TS, 4], [1, TS]]),
                        in1=_ap(tap, tap.offset + TS, [part, [2 * TS, 4], [1, TS]]),
                        op=OP.add)
                    nc.gpsimd.tensor_tensor(
                        out=t4[:, 0:2, :], in0=t4[:, 0:2, :], in1=t4[:, 2:4, :],
                        op=OP.add)
                    spart = t4[:, 0, :]
                    nc.gpsimd.tensor_tensor(out=spart, in0=spart,
                                            in1=t4[:, 1, :], op=OP.add)
                else:  # msz == 2
                    spart = t4[:, 0, :]
                    nc.gpsimd.tensor_tensor(out=spart, in0=traj[:, 0, :],
                                            in1=traj[:, 1, :], op=OP.add)
                if mc0 == 0:
                    nc.gpsimd.tensor_copy(out=S_b[:, 1:TS + 1], in_=spart)
                else:
                    nc.gpsimd.tensor_tensor(out=S_b[:, 1:TS + 1],
                                            in0=S_b[:, 1:TS + 1],
                                            in1=spart, op=OP.add)

            # ---- reads(b) on Pool ----
            d1 = rec1.tile([128, T], F32, tag="d1", bufs=1)
            nc.gpsimd.tensor_tensor(out=d1, in0=S_b[:, 0:T],
                                    in1=S_b[:, 1:T + 1], op=OP.subtract)
            nc.gpsimd.tensor_tensor(out=d1, in0=d1, in1=D50, op=OP.add)
            nc.gpsimd.tensor_tensor(out=d1, in0=d1, in1=a_bt[:, b, 0:T], op=OP.add)
            nc.gpsimd.tensor_tensor(out=rd_bt[:, b, 0:T], in0=d1,
                                    in1=er_bt[:, b, 0:T], op=OP.mult)

            # ---- phase 4(b) ----
            f_ps = finps.tile([128, 512], F32, tag="fps")
            nc.tensor.matmul(f_ps, Wf_r, reads_T[:, sl], start=True, stop=False)
            nc.tensor.matmul(f_ps, Wf_k, k_T[:, sl], start=False, stop=True)
            nc.scalar.activation(out=f_T[:, sl], in_=f_ps, func=AF.Tanh, bias=bf_col)
            hp2 = finps.tile([2, 512], F32, tag="fps")
            nc.tensor.matmul(hp2, Wab0, f_T[:, sl], start=True, stop=False)
            nc.tensor.matmul(hp2, W0d, k_T[:, sl], start=False, stop=True)
            ht = fin.tile([2, 512], BF16, tag="ht", bufs=1)
            nc.scalar.activation(out=ht, in_=hp2, func=AF.Tanh, bias=bias2)
            lg_ps = finps.tile([1, 512], F32, tag="fps")
            nc.tensor.matmul(lg_ps, comb, ht, start=True, stop=True)
            prob_row = fin.tile([1, 512], F32, tag="prob", bufs=1)
            nc.scalar.activation(out=prob_row, in_=lg_ps, func=AF.Sigmoid)
            nc.sync.dma_start(out=out_h[b:b + 1, :], in_=prob_row[0:1, 0:T])

        if dbg:
            nc.sync.dma_start(out=dbg["dbg_reads"][:, :], in_=reads_T)


_NC = None
LAST_RESULT = None


def _get_nc():
    global _NC
    if _NC is None:
        _NC = build_program()
    return _NC


def kernel(**inputs):
    global LAST_RESULT
    from concourse.bass_utils import run_bass_kernel_spmd

    nc = _get_nc()
    names = ["concept_seq", "correct_seq", "embed_key", "embed_value", "Mk", "Mv0",
             "We", "be", "Wa", "ba", "Wf", "bf", "Wab", "bab", "Wd", "bd"]
    full = {k: np.ascontiguousarray(np.asarray(inputs[k])) for k in names}
    in_maps = []
    for i in range(NCORES):
        m = dict(full)
        m["concept_seq"] = np.ascontiguousarray(full["concept_seq"][i * BL:(i + 1) * BL])
        m["correct_seq"] = np.ascontiguousarray(full["correct_seq"][i * BL:(i + 1) * BL])
        in_maps.append(m)
    res = run_bass_kernel_spmd(nc, in_maps, core_ids=list(range(NCORES)))
    LAST_RESULT = res
    return np.concatenate([res.results[i]["out"] for i in range(NCORES)], axis=0)


if __name__ == "__main__":
    nc = build_program()
    print("build ok")
